# revision 4
# baseline (speedup 1.0000x reference)
"""Trainium2 Bass kernel for nn_CensoredLoss_Sub.

reference:
    out = outputs.reshape(B, T, D)                     # D = 2
    loss1 = targets[:, :, 0:1] * log((1 - out) + eps)
    loss2 = targets[:, :, 1:2] * log(out + eps)
    loss  = sum((loss1 + loss2) * weights[:, :, None], axis=(0, 1))  # (D,)
    return -loss / (B * T)

Strategy (pure data-parallel over B across 8 cores):
  Each core gets B/8 rows. Flattened per-core streams:
    o: [128, FO]  (p-major flat view of the (B/8, T, 2) outputs shard)
    t: [128, FO]  targets shard, same layout
    w: [128, FO/2] weights shard (one per (b, t) pair)
  Per tile of F columns:
    ACT emits two interleaved log tiles (Ln table, scale/bias fused):
      M0 = [log(1-o0+eps), log(o0+eps)]  interleaved per pair  (d=0 terms)
      M1 = [log(1-o1+eps), log(o1+eps)]  interleaved per pair  (d=1 terms)
    DVE computes WT = t * w_bcast (w broadcast over the pair's two slots),
    then scalar_tensor_tensor multiplies WT with M0/M1 and accumulates the
    per-partition sum into one accumulator column per (iteration, d):
      sum(WT * M0) = sum_j w_j*(t0_j*log(1-o0_j+eps) + t1_j*log(o0_j+eps))
  Host sums the [128, 2*n_iter] per-core accumulators and applies -1/(B*T).
"""

import numpy as np

B, T, D = 16384, 512, 2
N_CORES = 8
EPS = 1e-8
P = 128

# per-core flattened sizes
FO = (B // N_CORES) * T * D // P  # o/t columns per partition = 16384
F_TILE = 2048                     # o-elems per partition per tile
N_ITER = FO // F_TILE

_compiled = {}


def _build():
    import concourse.mybir as mybir
    from concourse import bacc
    from concourse.tile import TileContext

    f32 = mybir.dt.float32
    Ln = mybir.ActivationFunctionType.Ln
    mult = mybir.AluOpType.mult

    nc = bacc.Bacc(
        "TRN2",
        target_bir_lowering=False,
        debug=False,
        num_devices=N_CORES,
    )
    o_d = nc.dram_tensor("o", [P, FO], f32, kind="ExternalInput").ap()
    t_d = nc.dram_tensor("t", [P, FO], f32, kind="ExternalInput").ap()
    w_d = nc.dram_tensor("w", [P, FO // 2], f32, kind="ExternalInput").ap()
    acc_d = nc.dram_tensor("acc", [P, 2 * N_ITER], f32, kind="ExternalOutput").ap()

    FP = F_TILE // 2  # pairs per partition per tile

    with TileContext(nc) as tc:
        with (
            tc.tile_pool(name="io", bufs=3) as io_pool,
            tc.tile_pool(name="mid", bufs=2) as mid_pool,
            tc.tile_pool(name="accp", bufs=1) as acc_pool,
        ):
            acc = acc_pool.tile([P, 2 * N_ITER], f32)
            bias_eps = acc_pool.tile([P, 1], f32)
            bias_1eps = acc_pool.tile([P, 1], f32)
            nc.vector.memset(bias_eps[:], EPS)
            nc.vector.memset(bias_1eps[:], 1.0 + EPS)
            for i in range(N_ITER):
                o = io_pool.tile([P, FP, 2], f32, tag="o")
                t = io_pool.tile([P, FP, 2], f32, tag="t")
                w = io_pool.tile([P, FP], f32, tag="w")
                sl = slice(i * F_TILE, (i + 1) * F_TILE)
                nc.sync.dma_start(out=o[:].rearrange("p f d -> p (f d)"), in_=o_d[:, sl])
                nc.sync.dma_start(out=t[:].rearrange("p f d -> p (f d)"), in_=t_d[:, sl])
                nc.sync.dma_start(out=w[:], in_=w_d[:, i * FP : (i + 1) * FP])

                m0 = mid_pool.tile([P, FP, 2], f32, tag="m0")
                m1 = mid_pool.tile([P, FP, 2], f32, tag="m1")
                # M0 even slot: log(1 - o0 + eps) = Ln(-1*o0 + (1+eps))
                nc.scalar.activation(m0[:, :, 0], o[:, :, 0], Ln, bias=bias_1eps[:], scale=-1.0)
                # M0 odd slot: log(o0 + eps)
                nc.scalar.activation(m0[:, :, 1], o[:, :, 0], Ln, bias=bias_eps[:], scale=1.0)
                nc.scalar.activation(m1[:, :, 0], o[:, :, 1], Ln, bias=bias_1eps[:], scale=-1.0)
                nc.scalar.activation(m1[:, :, 1], o[:, :, 1], Ln, bias=bias_eps[:], scale=1.0)

                # WT = t * w (w broadcast across the two slots of each pair)
                wt = mid_pool.tile([P, FP, 2], f32, tag="wt")
                wb = w[:].unsqueeze(-1).broadcast_to([P, FP, 2])
                nc.vector.tensor_mul(wt[:], t[:], wb)

                # acc[:, 2i+d] = sum(WT * Md) over the tile's free dims
                scr = mid_pool.tile([P, FP, 2], f32, tag="scr")
                nc.vector.scalar_tensor_tensor(
                    out=scr[:], in0=wt[:], scalar=1.0, in1=m0[:],
                    op0=mult, op1=mult,
                    accum_out=acc[:, 2 * i : 2 * i + 1],
                )
                nc.vector.scalar_tensor_tensor(
                    out=scr[:], in0=wt[:], scalar=1.0, in1=m1[:],
                    op0=mult, op1=mult,
                    accum_out=acc[:, 2 * i + 1 : 2 * i + 2],
                )
            nc.sync.dma_start(out=acc_d, in_=acc[:])
    nc.compile()
    return nc


def _get_nc():
    if "nc" not in _compiled:
        _compiled["nc"] = _build()
    return _compiled["nc"]


def make_in_maps(outputs, targets, weights):
    rows = B // N_CORES
    in_maps = []
    for c in range(N_CORES):
        sh = slice(c * rows, (c + 1) * rows)
        in_maps.append(
            {
                "o": np.ascontiguousarray(outputs[sh]).reshape(P, FO),
                "t": np.ascontiguousarray(targets[sh]).reshape(P, FO),
                "w": np.ascontiguousarray(weights[sh]).reshape(P, FO // 2),
            }
        )
    return in_maps


def run_raw(in_maps, **kw):
    from concourse import bass_utils

    nc = _get_nc()
    return bass_utils.run_bass_kernel_spmd(
        nc, in_maps, core_ids=list(range(N_CORES)), **kw
    )


def finish(results) -> np.ndarray:
    total = np.zeros(2, dtype=np.float64)
    for r in results:
        a = r["acc"].astype(np.float64)  # [P, 2*N_ITER]
        total[0] += a[:, 0::2].sum()
        total[1] += a[:, 1::2].sum()
    return (-total / (B * T)).astype(np.float32)


def kernel(outputs: np.ndarray, targets: np.ndarray, weights: np.ndarray) -> np.ndarray:
    res = run_raw(make_in_maps(outputs, targets, weights))
    return finish(res.results)
